# revision 6
# baseline (speedup 1.0000x reference)
"""Trainium2 Bass kernel for nn_LoRA_42374147342669 (moe_routing).

Math (TOPK=1 => renormalized routing weight == 1.0 exactly):
    out = x @ W.T + b + (x @ A[:r].T) @ B[:, :r].T
    where r = RANKS[argmax(x @ gate_w.T + gate_b)] per token.

Key identity: with u = x @ A.T (full rank 64),
    (x @ A[:r].T) @ B[:, :r].T == (u * mask_r) @ B.T,  mask_r = [1]*r + [0]*(64-r)
(adding exact zeros to the contraction), so one masked rank-64 matmul
replaces the 4 per-rank matmuls.

Sharding: data-parallel over the 8192 tokens across 8 NeuronCores
(1024 tokens/core); W, A, B, gate replicated.

Per-core pipeline (all heavy matmuls in float32r ~= fp32-accurate):
  1. gate+u pass: stationary [A_hi | g_hi | g_lo] bf16 x moving x_hi bf16
     plus [A_hi | g_hi | 0] x x_lo -> u = (xh+xl)@A_hi and fp32-class
     gate scores (bf16 hi/lo split; only the xl@gl term is dropped).
  2. routing: transpose scores -> per-token argmax -> one-hot ->
     mask[64,n] = M64 @ onehot (PE matmul), u_masked = u * mask.
  3. main loop over 16 output chunks of 256: PSUM accumulates
     lora+bias (one K=65 matmul vs [B.T | b]) + 32 K=128 f32r matmuls
     against W.T; single evacuation to the output.
All layout transforms (transposes/tiling) are host-side numpy.
"""
import sys
sys.path.insert(0, '/opt/trn_rl_repo')
import numpy as np
import ml_dtypes

P = 128
NCORES = 8
NT = 1024            # tokens per core
KT = 32              # k tiles of 128 over d_in=4096
D = 4096
OC = 256             # output chunk width
NOC = 16             # number of output chunks
RANKS = (8, 16, 32, 64)

_CACHE = {}


def _build(nrepeat=1):
    import concourse.bacc as bacc
    import concourse.mybir as mybir
    import concourse.tile as tile
    from concourse.masks import make_identity

    F32 = mybir.dt.float32
    F32R = mybir.dt.float32r
    BF16 = mybir.dt.bfloat16
    AF = mybir.ActivationFunctionType

    nc = bacc.Bacc("TRN2", target_bir_lowering=False)
    xt_d = nc.dram_tensor("xt", [P, KT, NT], F32R, kind="ExternalInput")
    xh_d = nc.dram_tensor("xh", [KT, P, NT], BF16, kind="ExternalInput")
    xl_d = nc.dram_tensor("xl", [KT, P, NT], BF16, kind="ExternalInput")
    wt_d = nc.dram_tensor("wt", [NOC, 4, P, KT // 4, OC], F32R, kind="ExternalInput")
    st1_d = nc.dram_tensor("st1", [P, KT, 72], BF16, kind="ExternalInput")
    st2_d = nc.dram_tensor("st2", [P, KT, 72], BF16, kind="ExternalInput")
    btb_d = nc.dram_tensor("btb", [NOC, 65, OC], F32R, kind="ExternalInput")
    gb_d = nc.dram_tensor("gb", [4, 1], F32, kind="ExternalInput")
    m64_d = nc.dram_tensor("m64", [4, 64], BF16, kind="ExternalInput")
    ones_d = nc.dram_tensor("ones", [1, NT], F32R, kind="ExternalInput")
    out_d = nc.dram_tensor("out", [8, NOC, P, OC], F32, kind="ExternalOutput")

    with tile.TileContext(nc) as tc:
        with tc.tile_pool(name="big", bufs=1) as big, \
             tc.tile_pool(name="wtp", bufs=5) as wtp, \
             tc.tile_pool(name="btp", bufs=2) as btp, \
             tc.tile_pool(name="gxp", bufs=2) as gxp, \
             tc.tile_pool(name="sm", bufs=1) as sm, \
             tc.tile_pool(name="evp", bufs=2) as evp, \
             tc.tile_pool(name="ps", bufs=1, space="PSUM") as ps:
            # constants
            ident4 = big.tile([4, 4], F32)
            make_identity(nc, ident4)
            ident_bf = big.tile([P, P], BF16)
            make_identity(nc, ident_bf)
            ST1 = big.tile([P, KT, 72], BF16)
            ST2 = big.tile([P, KT, 72], BF16)
            M64 = big.tile([4, 64], BF16)
            GB = big.tile([4, 1], F32)
            nc.sync.dma_start(ST1[:], st1_d[:])
            nc.sync.dma_start(ST2[:], st2_d[:])
            nc.sync.dma_start(M64[:], m64_d[:])
            nc.sync.dma_start(GB[:], gb_d[:])

            XT = big.tile([P, KT, NT], F32R)
            nc.sync.dma_start(XT[:], xt_d[:])
            UM = big.tile([65, NT], F32R)
            nc.sync.dma_start(UM[64:65, :], ones_d[:])

            def body():
                # ---- phase 1: gate scores + u ----
                psu = [ps.tile([72, 512], F32, name=f"psu{t}", tag=f"psu{t}")
                       for t in range(2)]
                for k in range(KT):
                    xh_t = gxp.tile([P, NT], BF16, name="xh_t", tag="xh_t")
                    nc.sync.dma_start(xh_t[:], xh_d[k])
                    xl_t = gxp.tile([P, NT], BF16, name="xl_t", tag="xl_t")
                    nc.sync.dma_start(xl_t[:], xl_d[k])
                    for t in range(2):
                        nsl = slice(t * 512, (t + 1) * 512)
                        nc.tensor.matmul(psu[t][:], ST1[:, k, :], xh_t[:, nsl],
                                         start=(k == 0), stop=False)
                        nc.tensor.matmul(psu[t][:], ST2[:, k, :], xl_t[:, nsl],
                                         start=False, stop=(k == KT - 1))
                # evacuate u rows and score rows
                stage = sm.tile([72, NT], F32, name="stage", tag="stage")
                scT = sm.tile([4, NT], F32, name="scT", tag="scT")
                scTb = sm.tile([4, NT], F32, name="scTb", tag="scTb")
                for t in range(2):
                    nsl = slice(t * 512, (t + 1) * 512)
                    nc.vector.tensor_copy(UM[:64, nsl], psu[t][:64, :])
                    nc.vector.tensor_copy(stage[64:72, nsl], psu[t][64:72, :])
                # partition-shift rows 64:68 / 68:72 down to 0:4 via sbuf-sbuf DMA
                nc.sync.dma_start(scT[:], stage[64:68, :])
                nc.sync.dma_start(scTb[:], stage[68:72, :])
                nc.vector.tensor_tensor(scT[:], scT[:], scTb[:],
                                        mybir.AluOpType.add)
                # + gate bias (per-partition)
                nc.scalar.activation(scT[:], scT[:], AF.Identity,
                                     bias=GB[:], scale=1.0)

                # ---- phase 2: routing -> mask -> u_masked ----
                onehotT = sm.tile([4, NT], BF16, name="onehotT", tag="onehotT")
                for m in range(8):
                    msl = slice(m * P, (m + 1) * P)
                    ps_s = ps.tile([P, 4], F32, name="ps_s", tag="ps_s")
                    nc.tensor.transpose(ps_s[:], scT[:, msl], ident4[:])
                    rmax = sm.tile([P, 1], F32, name="rmax", tag="rmax", bufs=2)
                    nc.vector.tensor_reduce(rmax[:], ps_s[:], mybir.AxisListType.X,
                                            mybir.AluOpType.max)
                    oh = sm.tile([P, 4], BF16, name="oh", tag="oh", bufs=2)
                    nc.vector.tensor_tensor(oh[:], ps_s[:],
                                            rmax[:].to_broadcast([P, 4]),
                                            mybir.AluOpType.is_ge)
                    ps_oh = ps.tile([4, P], BF16, name="ps_oh", tag="ps_oh")
                    nc.tensor.transpose(ps_oh[:], oh[:], ident_bf[:])
                    nc.vector.tensor_copy(onehotT[:, msl], ps_oh[:])
                for t in range(2):
                    nsl = slice(t * 512, (t + 1) * 512)
                    ps_m = ps.tile([64, 512], F32, name="ps_m", tag="ps_m")
                    nc.tensor.matmul(ps_m[:], M64[:], onehotT[:, nsl],
                                     start=True, stop=True)
                    nc.vector.tensor_tensor(UM[:64, nsl], UM[:64, nsl], ps_m[:],
                                            mybir.AluOpType.mult)

                # ---- phase 3: main GEMM + lora + bias ----
                for oc in range(NOC):
                    wts = []
                    for h in range(4):
                        wth = wtp.tile([P, KT // 4, OC], F32R, name="wth", tag="wth")
                        nc.sync.dma_start(wth[:], wt_d[oc, h])
                        wts.append(wth)
                    BT = btp.tile([65, OC], F32R, name="BT", tag="BT")
                    nc.sync.dma_start(BT[:], btb_d[oc])
                    for m in range(8):
                        msl = slice(m * P, (m + 1) * P)
                        po = ps.tile([P, OC], F32, name="po", tag="po", bufs=2)
                        nc.tensor.matmul(po[:], UM[:, msl], BT[:],
                                         start=True, stop=False)
                        for k in range(KT):
                            nc.tensor.matmul(po[:], XT[:, k, msl],
                                             wts[k // 8][:, k % 8, :],
                                             start=False, stop=(k == KT - 1))
                        ev = evp.tile([P, OC], F32, name="ev", tag="ev")
                        nc.vector.tensor_copy(ev[:], po[:])
                        nc.sync.dma_start(out_d[m, oc], ev[:])

            if nrepeat == 1:
                body()
            else:
                with tc.For_i(0, nrepeat) as _i:
                    body()
    nc.compile()
    return nc


def _prep_shared(W, b, A, B, gate_w, gate_b):
    bf16 = ml_dtypes.bfloat16
    f32 = np.float32
    # wt: [NOC, 2, P, KT//2, OC];  wt[oc,h,p,kh,j] = W[oc*OC+j, (h*16+kh)*128+p]
    wt = np.ascontiguousarray(
        W.reshape(NOC, OC, 4, KT // 4, P).transpose(0, 2, 4, 3, 1), dtype=f32)
    Ah = A.astype(bf16)
    gh = gate_w.astype(bf16)
    gl = (gate_w - gh.astype(f32)).astype(bf16)
    st1_full = np.concatenate([Ah, gh, gl], axis=0)               # [72, 4096] bf16
    st2_full = np.concatenate(
        [Ah, gh, np.zeros((4, D), bf16)], axis=0)
    # [P, KT, 72]; st[p,k,j] = full[j, k*128+p]
    st1 = np.ascontiguousarray(
        st1_full.T.reshape(KT, P, 72).transpose(1, 0, 2))
    st2 = np.ascontiguousarray(
        st2_full.T.reshape(KT, P, 72).transpose(1, 0, 2))
    btb_full = np.concatenate([B.T, b[None, :]], axis=0)          # [65, 4096]
    btb = np.ascontiguousarray(
        btb_full.reshape(65, NOC, OC).transpose(1, 0, 2), dtype=f32)
    m64 = np.zeros((4, 64), np.float32)
    for e, r in enumerate(RANKS):
        m64[e, :r] = 1.0
    return {
        "wt": wt,
        "st1": st1,
        "st2": st2,
        "btb": btb,
        "gb": np.ascontiguousarray(gate_b.reshape(4, 1), dtype=f32),
        "m64": m64.astype(bf16),
        "ones": np.ones((1, NT), f32),
    }


def _prep_percore(hidden_states):
    bf16 = ml_dtypes.bfloat16
    f32 = np.float32
    x2 = np.asarray(hidden_states, dtype=f32).reshape(NCORES * NT, D)
    # [c, p, k, n] with x2[c*NT+n, k*128+p]
    xt_all = np.ascontiguousarray(
        x2.reshape(NCORES, NT, KT, P).transpose(0, 3, 2, 1))
    xkpn = xt_all.transpose(0, 2, 1, 3)                    # [c, k, p, n] view
    xh_all = np.ascontiguousarray(xkpn).astype(bf16)
    xl_all = (xkpn - xh_all.astype(f32)).astype(bf16)
    maps = []
    for c in range(NCORES):
        maps.append({
            "xt": xt_all[c],
            "xh": xh_all[c],
            "xl": np.ascontiguousarray(xl_all[c]),
        })
    return maps


def kernel(hidden_states, W, b, A, B, gate_w, gate_b):
    from concourse import bass_utils

    hidden_states = np.asarray(hidden_states, np.float32)
    W = np.asarray(W, np.float32)
    b = np.asarray(b, np.float32)
    A = np.asarray(A, np.float32)
    B = np.asarray(B, np.float32)
    gate_w = np.asarray(gate_w, np.float32)
    gate_b = np.asarray(gate_b, np.float32)

    if "nc" not in _CACHE:
        _CACHE["nc"] = _build(1)
    nc = _CACHE["nc"]

    shared = _prep_shared(W, b, A, B, gate_w, gate_b)
    percore = _prep_percore(hidden_states)
    in_maps = [{**shared, **pc} for pc in percore]

    res = bass_utils.run_bass_kernel_spmd(
        nc, in_maps, core_ids=list(range(NCORES)))

    outs = []
    for c in range(NCORES):
        o = res.results[c]["out"]                     # [8, NOC, P, OC]
        outs.append(o.transpose(0, 2, 1, 3).reshape(NT, D))
    full = np.concatenate(outs, axis=0)
    bsz, seq, _ = hidden_states.shape
    return full.reshape(bsz, seq, D).astype(np.float32)
